# revision 15
# baseline (speedup 1.0000x reference)
"""Trainium2 Bass kernel for nn_Head (single-head causal self-attention).

Module:  q = x@Wq.T, k = x@Wk.T, v = x@Wv.T
         wei = softmax(causal_mask(q@k.T * E**-0.5))
         out = wei @ v
Shapes:  x [2048, 128, 192], Wq/Wk/Wv [192, 192] -> out [2048, 128, 192]

Strategy (pure data parallel over the batch dim, 8 cores x 256 batches):
  - Weight fold: wei = x @ A @ x.T with A = (Wq.T @ Wk) * SCALE, so only one
    projection ("g = x @ A") is needed for the attention logits.
  - Host pads the feature dim E 192 -> 256 with zeros and ships x transposed
    per-core as xt[e, b*T + t] in bf16.  The pad makes every K-chained
    matmul a full K=128 x K=128 pair: half-empty K=64 chain matmuls keep
    the PE's HAM activity monitor below its un-throttle threshold and lock
    the whole kernel at 1.2 GHz.
  - Logits are computed TRANSPOSED (weiT[k, q]) so the masked exp'd tile is
    directly the lhsT of the output matmul -- no PE transposes.
  - weiT's two K-halves are two independent single matmuls into separate
    PSUM banks; exp(wa+wb) = exp(wa)*exp(wb) merges them (ScalarE exp x2 +
    GpSimd mask + DVE multiply) -- single matmuls hide their LDWEIGHTS.
  - g and v keep PSUM accumulation chains but interleave the two chains of
    each quad across banks so chain LDWEIGHTS loads hide under the other
    chain's streaming.
  - Row sums ride along the output matmul via a ones-column on v; ScalarE /
    DVE normalize with per-partition reciprocals.
"""

import os
import sys

sys.path.insert(0, "/opt/trn_rl_repo")

import numpy as np
import ml_dtypes
from contextlib import ExitStack

import json

import concourse.bass as bass
import concourse.bass2jax as bass2jax
import concourse.mybir as mybir
import concourse.tile as tile
from concourse.bass_utils import (
    compile_bir_kernel as _orig_compile_bir_kernel,
    run_bass_kernel_spmd,
)

BF16 = mybir.dt.bfloat16
F32 = mybir.dt.float32
NPBF16 = ml_dtypes.bfloat16

B, T, E, H = 2048, 128, 192, 192
EP = 256                    # zero-padded feature dim (2 x 128)
NCORES = 8
NB = B // NCORES            # batches per core
SCALE = float(E) ** -0.5
G = 8                       # batches per DMA group
QUAD = 4                    # batches per pipeline stage
NGROUPS = NB // G


def _patch_tile_tail_drain():
    """Walrus rejects the TileContext tail Drain when it carries more than a
    couple of sem waits ("Too many sync wait commands").  Redistribute the
    waits onto single-wait SP nops emitted between the drain and barrier."""
    if getattr(tile.TileContext, "_tail_drain_patched", False):
        return

    def _drain_and_barrier(self, tick_clock, wait_clock):
        from concourse.tile import ScopedClock

        drain_inst = self.nc.sync.drain()
        wait_clock.add_sem_waits(
            drain_inst.ins, ScopedClock({None: tick_clock.global_clock})
        )
        waits = list(drain_inst.ins.sync_info.on_wait or [])
        if len(waits) > 1:
            drain_inst.ins.sync_info = mybir.SyncInfo(
                on_wait=[waits[0]], on_update=[]
            )
            for w in waits[1:]:
                nop = self.nc.sync.nop()
                nop.ins.sync_info = mybir.SyncInfo(on_wait=[w], on_update=[])
        self.nc.all_engine_barrier()
        assert self.sems is not None
        popped = self.nc._tile_sem_poison_stack.pop()
        assert popped is self._sem_poison
        self.nc.clear_and_free_semaphores(list(self.sems.allocated().values()))
        self.nc.all_engine_barrier()

    tile.TileContext._drain_and_barrier = _drain_and_barrier
    tile.TileContext._tail_drain_patched = True


def _split_multi_waits(bir_json: bytes) -> bytes:
    """This container's walrus supports only ONE sync-wait slot per
    instruction ("Too many sync wait commands").  Hoist extra waits onto
    single-wait NoOps inserted just before the instruction (same engine, so
    per-engine program order and blocking semantics are preserved)."""
    d = json.loads(bir_json)
    n = 0
    for f in d.get("functions", []):
        for bb in f.get("blocks", []):
            insts = bb.get("instructions", [])
            out = []
            changed = False
            for inst in insts:
                si = inst.get("sync_info")
                waits = (si.get("on_wait") or []) if si else []
                if len(waits) > 1:
                    changed = True
                    for w in waits[:-1]:
                        n += 1
                        out.append({
                            "debug": inst.get("debug"),
                            "engine": inst["engine"],
                            "ins": [],
                            "name": f"WSPLIT-{n}",
                            "opcode": "NoOp",
                            "outs": [],
                            "sync_info": {"on_update": [], "on_wait": [w]},
                        })
                    si["on_wait"] = [waits[-1]]
                out.append(inst)
            if changed:
                bb["instructions"] = out
    if n == 0:
        return bir_json
    return json.dumps(d).encode()


def _patched_compile_bir_kernel(bir_json, tmpdir, neff_name="file.neff"):
    if isinstance(bir_json, str):
        bir_json = bir_json.encode()
    return _orig_compile_bir_kernel(_split_multi_waits(bir_json), tmpdir, neff_name)


bass2jax.compile_bir_kernel = _patched_compile_bir_kernel


def build_nc(nb=NB):
    _patch_tile_tail_drain()
    nc = bass.Bass(trn_type="TRN2")

    xt = nc.dram_tensor("xt", [EP, nb * T], BF16, kind="ExternalInput")
    a = nc.dram_tensor("a", [EP, EP], BF16, kind="ExternalInput")
    wvt = nc.dram_tensor("wvt", [EP, H], BF16, kind="ExternalInput")
    # Output laid out [t, b, h] for contiguous DMA; host transposes.
    o = nc.dram_tensor("o", [T, nb, H], BF16, kind="ExternalOutput")

    nq = nb // QUAD
    Exp = mybir.ActivationFunctionType.Exp

    with tile.TileContext(nc) as tc, ExitStack() as ctx:
        singles = ctx.enter_context(tc.tile_pool(name="singles", bufs=1))
        px = ctx.enter_context(tc.tile_pool(name="px", bufs=4))
        pgsb = ctx.enter_context(tc.tile_pool(name="pgsb", bufs=4))
        ppa = ctx.enter_context(tc.tile_pool(name="ppa", bufs=2))
        ppb = ctx.enter_context(tc.tile_pool(name="ppb", bufs=2))
        ppm = ctx.enter_context(tc.tile_pool(name="ppm", bufs=4))
        pvsb = ctx.enter_context(tc.tile_pool(name="pvsb", bufs=8))
        psr = ctx.enter_context(tc.tile_pool(name="psr", bufs=6))
        posb = ctx.enter_context(tc.tile_pool(name="posb", bufs=2))

        pg = ctx.enter_context(tc.tile_pool(name="pg", bufs=1, space="PSUM"))
        pwa = ctx.enter_context(tc.tile_pool(name="pwa", bufs=1, space="PSUM"))
        pwb = ctx.enter_context(tc.tile_pool(name="pwb", bufs=1, space="PSUM"))
        pv = ctx.enter_context(tc.tile_pool(name="pv", bufs=2, space="PSUM"))
        po = ctx.enter_context(tc.tile_pool(name="po", bufs=2, space="PSUM"))

        # Constants: A (lhsT for gT), WvT (rhs for v); hi halves zero-padded
        # to full 128 partitions.
        a_lo = singles.tile([128, EP], BF16, tag="a_lo")
        a_hi = singles.tile([128, EP], BF16, tag="a_hi")
        nc.sync.dma_start(out=a_lo, in_=a[0:128, :])
        nc.sync.dma_start(out=a_hi, in_=a[128:256, :])
        wvt_lo = singles.tile([128, H], BF16, tag="wvt_lo")
        wvt_hi = singles.tile([128, H], BF16, tag="wvt_hi")
        nc.sync.dma_start(out=wvt_lo, in_=wvt[0:128, :])
        nc.sync.dma_start(out=wvt_hi, in_=wvt[128:256, :])

        # Software pipeline over quads: iteration Q emits
        #   gT(Q), v(Q)  ->  weiT(Q-1) + exp + mask  ->  o(Q-2)
        x_tiles = {}     # group -> (xlo, xhi)
        gsb_t = {}       # Q -> gsb ([128,1024]: gT_lo | gT_hi)
        pm_t = {}        # Q -> masked exp'd weiT (PmT)
        vsb_t = {}       # (Q, pr) -> v_sb pair
        osb_t = {}       # group -> o_sb

        for Q in range(nq + 2):
            if Q < nq:
                g = Q * QUAD // G
                if (Q * QUAD) % G == 0:
                    gcol = g * G * T
                    xlo = px.tile([128, G * T], BF16, tag="xlo")
                    xhi = px.tile([128, G * T], BF16, tag="xhi")
                    nc.sync.dma_start(out=xlo, in_=xt[0:128, gcol : gcol + G * T])
                    nc.sync.dma_start(out=xhi, in_=xt[128:256, gcol : gcol + G * T])
                    x_tiles[g] = (xlo, xhi)
                xlo, xhi = x_tiles[g]
                qs = (Q * QUAD * T) % (G * T)
                qcols = slice(qs, qs + QUAD * T)

                # gT = A.T @ xT for 4 batches; the lo-rows chain (bank0) and
                # hi-rows chain (bank1) are interleaved so each chain's
                # LDWEIGHTS hides under the other chain's streaming.
                gt = pg.tile([128, 1024], F32, tag="gt")
                v_ps0 = pv.tile([128, 2, 256], F32, tag="v_ps", name="v_ps0")
                v_ps1 = pv.tile([128, 2, 256], F32, tag="v_ps", name="v_ps1")
                # g chains woven between v chain halves: every chain-stop
                # LDWEIGHTS gets a preceding stream to hide under.
                nc.tensor.matmul(gt[:, 0:512], a_lo[:, 0:128], xlo[:, qcols],
                                 start=True, stop=False)
                nc.tensor.matmul(gt[:, 512:1024], a_lo[:, 128:256],
                                 xlo[:, qcols], start=True, stop=False)
                b0 = qs
                b1 = qs + 2 * T
                nc.tensor.matmul(v_ps0[:, 0, 0:H], xlo[:, b0 : b0 + T],
                                 wvt_lo, start=True, stop=False)
                nc.tensor.matmul(v_ps1[:, 0, 0:H], xlo[:, b1 : b1 + T],
                                 wvt_lo, start=True, stop=False)
                nc.tensor.matmul(gt[:, 0:512], a_hi[:, 0:128], xhi[:, qcols],
                                 start=False, stop=True)
                nc.tensor.matmul(v_ps0[:, 0, 0:H], xhi[:, b0 : b0 + T],
                                 wvt_hi, start=False, stop=True)
                nc.tensor.matmul(gt[:, 512:1024], a_hi[:, 128:256],
                                 xhi[:, qcols], start=False, stop=True)
                nc.tensor.matmul(v_ps1[:, 0, 0:H], xhi[:, b1 : b1 + T],
                                 wvt_hi, start=False, stop=True)
                gt_t = gt

                for jj in (1,):
                    b0 = qs + jj * T
                    b1 = qs + (2 + jj) * T
                    nc.tensor.matmul(v_ps0[:, jj, 0:H], xlo[:, b0 : b0 + T],
                                     wvt_lo, start=True, stop=False)
                    nc.tensor.matmul(v_ps1[:, jj, 0:H], xlo[:, b1 : b1 + T],
                                     wvt_lo, start=True, stop=False)
                    nc.tensor.matmul(v_ps0[:, jj, 0:H], xhi[:, b0 : b0 + T],
                                     wvt_hi, start=False, stop=True)
                    nc.tensor.matmul(v_ps1[:, jj, 0:H], xhi[:, b1 : b1 + T],
                                     wvt_hi, start=False, stop=True)
                gsb = pgsb.tile([128, 1024], BF16, tag="gsb")
                if Q < 4:
                    # padded wei chains read gsb rows 64:128 of the hi half;
                    # zero each rotating pool buffer once
                    nc.gpsimd.memset(gsb[64:128, 512:1024], 0.0)
                nc.vector.tensor_copy(out=gsb[0:64, 512:1024],
                                      in_=gt[0:64, 512:1024])
                gsb_t[Q] = gsb
                for pr, v_ps in ((0, v_ps0), (1, v_ps1)):
                    v_sb = pvsb.tile([128, 2, 200], BF16, tag="v_sb")
                    nc.gpsimd.memset(v_sb[:, :, H : H + 1], 1.0)
                    nc.vector.tensor_copy(out=v_sb[:, :, 0:H],
                                          in_=v_ps[:, :, 0:H])
                    vsb_t[(Q, pr)] = v_sb

            # weiT(P) = xT.T @ gT as full-K (padded) chains: batch pair 0
            # in bank A, pair 1 in bank B, chains interleaved across banks
            # so each chain's LDWEIGHTS hides under the other's streaming.
            if 1 <= Q <= nq:
                P = Q - 1
                pg_ = P * QUAD // G
                xlo_p, xhi_p = x_tiles[pg_]
                ps_ = (P * QUAD * T) % (G * T)
                gsb = gsb_t.pop(P)
                wa = pwa.tile([128, 2, T], F32, tag="wa")
                wb = pwb.tile([128, 2, T], F32, tag="wb")
                for jj in range(2):
                    ka = ps_ + jj * T
                    kb = ps_ + (2 + jj) * T
                    ja = slice(jj * T, (jj + 1) * T)
                    jb = slice((2 + jj) * T, (3 + jj) * T)
                    nc.tensor.matmul(wa[:, jj, :], xlo_p[:, ka : ka + T],
                                     gsb[:, ja], start=True, stop=False)
                    nc.tensor.matmul(wb[:, jj, :], xlo_p[:, kb : kb + T],
                                     gsb[:, jb], start=True, stop=False)
                    nc.tensor.matmul(wa[:, jj, :], xhi_p[:, ka : ka + T],
                                     gsb[:, 512 + jj * T : 512 + (jj + 1) * T],
                                     start=False, stop=True)
                    nc.tensor.matmul(wb[:, jj, :], xhi_p[:, kb : kb + T],
                                     gsb[:, 512 + (2 + jj) * T : 512 + (3 + jj) * T],
                                     start=False, stop=True)
                pA = ppa.tile([128, 2, T], BF16, tag="pA")
                nc.scalar.activation(out=pA, in_=wa, func=Exp)
                pB = ppb.tile([128, 2, T], BF16, tag="pB")
                nc.scalar.activation(out=pB, in_=wb, func=Exp)
                # causal mask: keep where q >= k  (k = partition index)
                pmA = ppm.tile([128, 2, T], BF16, tag="pmA", name="pmA")
                nc.gpsimd.affine_select(
                    out=pmA, in_=pA,
                    compare_op=mybir.AluOpType.is_ge,
                    fill=0.0, base=0, channel_multiplier=-1,
                    pattern=[[0, 2], [1, T]],
                )
                pmB = ppm.tile([128, 2, T], BF16, tag="pmB", name="pmB")
                nc.gpsimd.affine_select(
                    out=pmB, in_=pB,
                    compare_op=mybir.AluOpType.is_ge,
                    fill=0.0, base=0, channel_multiplier=-1,
                    pattern=[[0, 2], [1, T]],
                )
                pm_t[P] = (pmA, pmB)
                if Q < nq:
                    nc.scalar.copy(out=gsb_t[Q][:, 0:512], in_=gt_t[:, 0:512])

            # o(O) = PmT.T @ v_ext ; col H = softmax denominator
            if Q >= 2:
                O_ = Q - 2
                pmA, pmB = pm_t.pop(O_)
                gb = O_ * QUAD // G
                ob0 = (O_ * QUAD) % G
                if ob0 == 0:
                    osb_t[gb] = posb.tile([128, G, H], BF16, tag="o_sb",
                                          name="o_sb")
                o_sb = osb_t[gb]
                for pr in range(QUAD // 2):
                    v_sb = vsb_t.pop((O_, pr))
                    pm = pmA if pr == 0 else pmB
                    o_ps = po.tile([128, 2, 256], F32, tag="o_ps")
                    for jj in range(2):
                        nc.tensor.matmul(o_ps[:, jj, 0 : H + 1], pm[:, jj, :],
                                         v_sb[:, jj, 0 : H + 1],
                                         start=True, stop=True)
                    r = psr.tile([128, 2], F32, tag="r")
                    nc.vector.reciprocal(out=r, in_=o_ps[:, :, H])
                    ob = ob0 + pr * 2
                    nc.vector.tensor_scalar_mul(
                        out=o_sb[:, ob, :], in0=o_ps[:, 0, 0:H],
                        scalar1=r[:, 0:1],
                    )
                    nc.scalar.mul(out=o_sb[:, ob + 1, :], in_=o_ps[:, 1, 0:H],
                                  mul=r[:, 1:2])
                if ob0 + QUAD == G:
                    nc.sync.dma_start(
                        out=o[:, gb * G : (gb + 1) * G, :], in_=o_sb
                    )
                    del osb_t[gb]

            # iteration 0 has no wei block; emit its glo copy here
            if Q == 0:
                nc.scalar.copy(out=gsb_t[0][:, 0:512], in_=gt_t[:, 0:512])
    return nc


_cached = {}


def _get_nc(nb):
    if nb not in _cached:
        _cached[nb] = build_nc(nb)
    return _cached[nb]


def prep_inputs(x, Wq, Wk, Wv, nb=NB, ncores=NCORES):
    """Host-side sharding + layout/dtype prep + weight folding + zero-pad."""
    x = np.asarray(x, dtype=np.float32)
    A = (np.asarray(Wq, np.float32).T @ np.asarray(Wk, np.float32)) * SCALE
    a_bf = np.zeros((EP, EP), dtype=NPBF16)
    a_bf[0:E, 0:E] = A.astype(NPBF16)
    wvt_bf = np.zeros((EP, H), dtype=NPBF16)
    wvt_bf[0:E] = np.ascontiguousarray(np.asarray(Wv, np.float32).T).astype(NPBF16)
    in_maps = []
    for c in range(ncores):
        shard = x[c * nb : (c + 1) * nb]                      # [nb, T, E]
        xt = np.zeros((EP, nb * T), dtype=NPBF16)
        xt[0:E] = (
            np.ascontiguousarray(shard.transpose(2, 0, 1))
            .reshape(E, nb * T)
            .astype(NPBF16)
        )
        in_maps.append({"xt": xt, "a": a_bf, "wvt": wvt_bf})
    return in_maps


def kernel(x, Wq, Wk, Wv, _trace=False):
    nc = _get_nc(NB)
    in_maps = prep_inputs(x, Wq, Wk, Wv)
    res = run_bass_kernel_spmd(
        nc, in_maps, core_ids=list(range(NCORES)), trace=_trace
    )
    # o is [T, nb, H] per core; transpose to [nb, T, H] and concat.
    out = np.concatenate(
        [np.asarray(res.results[c]["o"], dtype=np.float32).transpose(1, 0, 2)
         for c in range(NCORES)], axis=0
    )
    out = np.ascontiguousarray(out, dtype=np.float32)
    if _trace:
        kernel.last_result = res
    return out


# revision 16
# speedup vs baseline: 1.0231x; 1.0231x over previous
"""Trainium2 Bass kernel for nn_Head (single-head causal self-attention).

Module:  q = x@Wq.T, k = x@Wk.T, v = x@Wv.T
         wei = softmax(causal_mask(q@k.T * E**-0.5))
         out = wei @ v
Shapes:  x [2048, 128, 192], Wq/Wk/Wv [192, 192] -> out [2048, 128, 192]

Strategy (pure data parallel over the batch dim, 8 cores x 256 batches):
  - Weight fold: wei = x @ A @ x.T with A = (Wq.T @ Wk) * SCALE, so only one
    projection ("g = x @ A") is needed for the attention logits.
  - Host pads the feature dim E 192 -> 256 with zeros and ships x transposed
    per-core as xt[e, b*T + t] in bf16.  The pad makes every K-chained
    matmul a full K=128 x K=128 pair: half-empty K=64 chain matmuls keep
    the PE's HAM activity monitor below its un-throttle threshold and lock
    the whole kernel at 1.2 GHz.
  - Logits are computed TRANSPOSED (weiT[k, q]) so the masked exp'd tile is
    directly the lhsT of the output matmul -- no PE transposes.
  - weiT's two K-halves are two independent single matmuls into separate
    PSUM banks; exp(wa+wb) = exp(wa)*exp(wb) merges them (ScalarE exp x2 +
    GpSimd mask + DVE multiply) -- single matmuls hide their LDWEIGHTS.
  - g and v keep PSUM accumulation chains but interleave the two chains of
    each quad across banks so chain LDWEIGHTS loads hide under the other
    chain's streaming.
  - Row sums ride along the output matmul via a ones-column on v; ScalarE /
    DVE normalize with per-partition reciprocals.
"""

import os
import sys

sys.path.insert(0, "/opt/trn_rl_repo")

import numpy as np
import ml_dtypes
from contextlib import ExitStack

import json

import concourse.bass as bass
import concourse.bass2jax as bass2jax
import concourse.mybir as mybir
import concourse.tile as tile
from concourse.bass_utils import (
    compile_bir_kernel as _orig_compile_bir_kernel,
    run_bass_kernel_spmd,
)

BF16 = mybir.dt.bfloat16
F32 = mybir.dt.float32
NPBF16 = ml_dtypes.bfloat16

B, T, E, H = 2048, 128, 192, 192
EP = 256                    # zero-padded feature dim (2 x 128)
NCORES = 8
NB = B // NCORES            # batches per core
SCALE = float(E) ** -0.5
G = 8                       # batches per DMA group
QUAD = 4                    # batches per pipeline stage
NGROUPS = NB // G


def _patch_tile_tail_drain():
    """Walrus rejects the TileContext tail Drain when it carries more than a
    couple of sem waits ("Too many sync wait commands").  Redistribute the
    waits onto single-wait SP nops emitted between the drain and barrier."""
    if getattr(tile.TileContext, "_tail_drain_patched", False):
        return

    def _drain_and_barrier(self, tick_clock, wait_clock):
        from concourse.tile import ScopedClock

        drain_inst = self.nc.sync.drain()
        wait_clock.add_sem_waits(
            drain_inst.ins, ScopedClock({None: tick_clock.global_clock})
        )
        waits = list(drain_inst.ins.sync_info.on_wait or [])
        if len(waits) > 1:
            drain_inst.ins.sync_info = mybir.SyncInfo(
                on_wait=[waits[0]], on_update=[]
            )
            for w in waits[1:]:
                nop = self.nc.sync.nop()
                nop.ins.sync_info = mybir.SyncInfo(on_wait=[w], on_update=[])
        self.nc.all_engine_barrier()
        assert self.sems is not None
        popped = self.nc._tile_sem_poison_stack.pop()
        assert popped is self._sem_poison
        self.nc.clear_and_free_semaphores(list(self.sems.allocated().values()))
        self.nc.all_engine_barrier()

    tile.TileContext._drain_and_barrier = _drain_and_barrier
    tile.TileContext._tail_drain_patched = True


def _split_multi_waits(bir_json: bytes) -> bytes:
    """This container's walrus supports only ONE sync-wait slot per
    instruction ("Too many sync wait commands").  Hoist extra waits onto
    single-wait NoOps inserted just before the instruction (same engine, so
    per-engine program order and blocking semantics are preserved)."""
    d = json.loads(bir_json)
    n = 0
    for f in d.get("functions", []):
        for bb in f.get("blocks", []):
            insts = bb.get("instructions", [])
            out = []
            changed = False
            for inst in insts:
                si = inst.get("sync_info")
                waits = (si.get("on_wait") or []) if si else []
                if len(waits) > 1:
                    changed = True
                    for w in waits[:-1]:
                        n += 1
                        out.append({
                            "debug": inst.get("debug"),
                            "engine": inst["engine"],
                            "ins": [],
                            "name": f"WSPLIT-{n}",
                            "opcode": "NoOp",
                            "outs": [],
                            "sync_info": {"on_update": [], "on_wait": [w]},
                        })
                    si["on_wait"] = [waits[-1]]
                out.append(inst)
            if changed:
                bb["instructions"] = out
    if n == 0:
        return bir_json
    return json.dumps(d).encode()


def _patched_compile_bir_kernel(bir_json, tmpdir, neff_name="file.neff"):
    if isinstance(bir_json, str):
        bir_json = bir_json.encode()
    return _orig_compile_bir_kernel(_split_multi_waits(bir_json), tmpdir, neff_name)


bass2jax.compile_bir_kernel = _patched_compile_bir_kernel


def build_nc(nb=NB):
    _patch_tile_tail_drain()
    nc = bass.Bass(trn_type="TRN2")

    xt = nc.dram_tensor("xt", [EP, nb * T], BF16, kind="ExternalInput")
    a = nc.dram_tensor("a", [EP, EP], BF16, kind="ExternalInput")
    wvt = nc.dram_tensor("wvt", [EP, H], BF16, kind="ExternalInput")
    # Output laid out [t, b, h] for contiguous DMA; host transposes.
    o = nc.dram_tensor("o", [T, nb, H], BF16, kind="ExternalOutput")

    nq = nb // QUAD
    Exp = mybir.ActivationFunctionType.Exp

    with tile.TileContext(nc) as tc, ExitStack() as ctx:
        singles = ctx.enter_context(tc.tile_pool(name="singles", bufs=1))
        px = ctx.enter_context(tc.tile_pool(name="px", bufs=4))
        pgsb = ctx.enter_context(tc.tile_pool(name="pgsb", bufs=3))
        ppa = ctx.enter_context(tc.tile_pool(name="ppa", bufs=2))
        ppb = ctx.enter_context(tc.tile_pool(name="ppb", bufs=2))
        ppm = ctx.enter_context(tc.tile_pool(name="ppm", bufs=3))
        pvsb = ctx.enter_context(tc.tile_pool(name="pvsb", bufs=6))
        psr = ctx.enter_context(tc.tile_pool(name="psr", bufs=4))
        posb = ctx.enter_context(tc.tile_pool(name="posb", bufs=2))

        pg = ctx.enter_context(tc.tile_pool(name="pg", bufs=1, space="PSUM"))
        pwa = ctx.enter_context(tc.tile_pool(name="pwa", bufs=1, space="PSUM"))
        pwb = ctx.enter_context(tc.tile_pool(name="pwb", bufs=1, space="PSUM"))
        pv = ctx.enter_context(tc.tile_pool(name="pv", bufs=2, space="PSUM"))
        po = ctx.enter_context(tc.tile_pool(name="po", bufs=2, space="PSUM"))

        # Constants: A (lhsT for gT), WvT (rhs for v); hi halves zero-padded
        # to full 128 partitions.
        a_lo = singles.tile([128, EP], BF16, tag="a_lo")
        a_hi = singles.tile([128, EP], BF16, tag="a_hi")
        nc.sync.dma_start(out=a_lo, in_=a[0:128, :])
        nc.sync.dma_start(out=a_hi, in_=a[128:256, :])
        wvt_lo = singles.tile([128, H], BF16, tag="wvt_lo")
        wvt_hi = singles.tile([128, H], BF16, tag="wvt_hi")
        nc.sync.dma_start(out=wvt_lo, in_=wvt[0:128, :])
        nc.sync.dma_start(out=wvt_hi, in_=wvt[128:256, :])

        # Software pipeline over quads: iteration Q emits
        #   gT(Q), v(Q)  ->  weiT(Q-1) + exp + mask  ->  o(Q-2)
        x_tiles = {}     # group -> (xlo, xhi)
        gsb_t = {}       # Q -> gsb ([128,1024]: gT_lo | gT_hi)
        pm_t = {}        # Q -> masked exp'd weiT (PmT)
        vsb_t = {}       # (Q, pr) -> v_sb pair
        osb_t = {}       # group -> o_sb

        for Q in range(nq + 2):
            if Q < nq:
                g = Q * QUAD // G
                if (Q * QUAD) % G == 0:
                    gcol = g * G * T
                    xlo = px.tile([128, G * T], BF16, tag="xlo")
                    xhi = px.tile([128, G * T], BF16, tag="xhi")
                    nc.sync.dma_start(out=xlo, in_=xt[0:128, gcol : gcol + G * T])
                    nc.sync.dma_start(out=xhi, in_=xt[128:256, gcol : gcol + G * T])
                    x_tiles[g] = (xlo, xhi)
                xlo, xhi = x_tiles[g]
                qs = (Q * QUAD * T) % (G * T)
                qcols = slice(qs, qs + QUAD * T)

                # gT = A.T @ xT for 4 batches; the lo-rows chain (bank0) and
                # hi-rows chain (bank1) are interleaved so each chain's
                # LDWEIGHTS hides under the other chain's streaming.
                gt = pg.tile([128, 1024], F32, tag="gt")
                v_ps0 = pv.tile([128, 2, 256], F32, tag="v_ps", name="v_ps0")
                v_ps1 = pv.tile([128, 2, 256], F32, tag="v_ps", name="v_ps1")
                # g chains woven between v chain halves: every chain-stop
                # LDWEIGHTS gets a preceding stream to hide under.
                nc.tensor.matmul(gt[:, 0:512], a_lo[:, 0:128], xlo[:, qcols],
                                 start=True, stop=False)
                nc.tensor.matmul(gt[:, 512:1024], a_lo[:, 128:256],
                                 xlo[:, qcols], start=True, stop=False)
                b0 = qs
                b1 = qs + 2 * T
                nc.tensor.matmul(v_ps0[:, 0, 0:H], xlo[:, b0 : b0 + T],
                                 wvt_lo, start=True, stop=False)
                nc.tensor.matmul(v_ps1[:, 0, 0:H], xlo[:, b1 : b1 + T],
                                 wvt_lo, start=True, stop=False)
                nc.tensor.matmul(gt[:, 0:512], a_hi[:, 0:128], xhi[:, qcols],
                                 start=False, stop=True)
                nc.tensor.matmul(v_ps0[:, 0, 0:H], xhi[:, b0 : b0 + T],
                                 wvt_hi, start=False, stop=True)
                nc.tensor.matmul(gt[:, 512:1024], a_hi[:, 128:256],
                                 xhi[:, qcols], start=False, stop=True)
                nc.tensor.matmul(v_ps1[:, 0, 0:H], xhi[:, b1 : b1 + T],
                                 wvt_hi, start=False, stop=True)
                gt_t = gt

                for jj in (1,):
                    b0 = qs + jj * T
                    b1 = qs + (2 + jj) * T
                    nc.tensor.matmul(v_ps0[:, jj, 0:H], xlo[:, b0 : b0 + T],
                                     wvt_lo, start=True, stop=False)
                    nc.tensor.matmul(v_ps1[:, jj, 0:H], xlo[:, b1 : b1 + T],
                                     wvt_lo, start=True, stop=False)
                    nc.tensor.matmul(v_ps0[:, jj, 0:H], xhi[:, b0 : b0 + T],
                                     wvt_hi, start=False, stop=True)
                    nc.tensor.matmul(v_ps1[:, jj, 0:H], xhi[:, b1 : b1 + T],
                                     wvt_hi, start=False, stop=True)
                for pr, v_ps in ((0, v_ps0), (1, v_ps1)):
                    v_sb = pvsb.tile([128, 2, 200], BF16, tag="v_sb")
                    nc.gpsimd.memset(v_sb[:, :, H : H + 1], 1.0)
                    nc.vector.tensor_copy(out=v_sb[:, :, 0:H],
                                          in_=v_ps[:, :, 0:H])
                    vsb_t[(Q, pr)] = v_sb

            # weiT(P) = xT.T @ gT as full-K (padded) chains: batch pair 0
            # in bank A, pair 1 in bank B, chains interleaved across banks
            # so each chain's LDWEIGHTS hides under the other's streaming.
            if 1 <= Q <= nq:
                P = Q - 1
                pg_ = P * QUAD // G
                xlo_p, xhi_p = x_tiles[pg_]
                ps_ = (P * QUAD * T) % (G * T)
                gsb = gsb_t.pop(P)
                wa = pwa.tile([128, 2, T], F32, tag="wa")
                wb = pwb.tile([128, 2, T], F32, tag="wb")
                for jj in range(2):
                    ka = ps_ + jj * T
                    kb = ps_ + (2 + jj) * T
                    ja = slice(jj * T, (jj + 1) * T)
                    jb = slice((2 + jj) * T, (3 + jj) * T)
                    nc.tensor.matmul(wa[:, jj, :], xlo_p[:, ka : ka + T],
                                     gsb[:, ja], start=True, stop=False)
                    nc.tensor.matmul(wb[:, jj, :], xlo_p[:, kb : kb + T],
                                     gsb[:, jb], start=True, stop=False)
                    nc.tensor.matmul(wa[:, jj, :], xhi_p[:, ka : ka + T],
                                     gsb[:, 512 + jj * T : 512 + (jj + 1) * T],
                                     start=False, stop=True)
                    nc.tensor.matmul(wb[:, jj, :], xhi_p[:, kb : kb + T],
                                     gsb[:, 512 + (2 + jj) * T : 512 + (3 + jj) * T],
                                     start=False, stop=True)
                pA = ppa.tile([128, 2, T], BF16, tag="pA")
                nc.scalar.activation(out=pA, in_=wa, func=Exp)
                pB = ppb.tile([128, 2, T], BF16, tag="pB")
                nc.scalar.activation(out=pB, in_=wb, func=Exp)
                # causal mask: keep where q >= k  (k = partition index)
                pmA = ppm.tile([128, 2, T], BF16, tag="pmA", name="pmA")
                nc.gpsimd.affine_select(
                    out=pmA, in_=pA,
                    compare_op=mybir.AluOpType.is_ge,
                    fill=0.0, base=0, channel_multiplier=-1,
                    pattern=[[0, 2], [1, T]],
                )
                pmB = ppm.tile([128, 2, T], BF16, tag="pmB", name="pmB")
                nc.gpsimd.affine_select(
                    out=pmB, in_=pB,
                    compare_op=mybir.AluOpType.is_ge,
                    fill=0.0, base=0, channel_multiplier=-1,
                    pattern=[[0, 2], [1, T]],
                )
                pm_t[P] = (pmA, pmB)
                if Q < nq:
                    gsb2 = pgsb.tile([128, 1024], BF16, tag="gsb",
                                     name="gsb2")
                    if Q < 3:
                        # padded wei chains read gsb rows 64:128 of the hi
                        # half; zero each rotating pool buffer once
                        nc.gpsimd.memset(gsb2[64:128, 512:1024], 0.0)
                    nc.scalar.copy(out=gsb2[:, 0:512], in_=gt_t[:, 0:512])
                    nc.vector.tensor_copy(out=gsb2[0:64, 512:1024],
                                          in_=gt_t[0:64, 512:1024])
                    gsb_t[Q] = gsb2

            # o(O) = PmT.T @ v_ext ; col H = softmax denominator
            if Q >= 2:
                O_ = Q - 2
                pmA, pmB = pm_t.pop(O_)
                gb = O_ * QUAD // G
                ob0 = (O_ * QUAD) % G
                if ob0 == 0:
                    osb_t[gb] = posb.tile([128, G, H], BF16, tag="o_sb",
                                          name="o_sb")
                o_sb = osb_t[gb]
                for pr in range(QUAD // 2):
                    v_sb = vsb_t.pop((O_, pr))
                    pm = pmA if pr == 0 else pmB
                    o_ps = po.tile([128, 2, 256], F32, tag="o_ps")
                    for jj in range(2):
                        nc.tensor.matmul(o_ps[:, jj, 0 : H + 1], pm[:, jj, :],
                                         v_sb[:, jj, 0 : H + 1],
                                         start=True, stop=True)
                    r = psr.tile([128, 2], F32, tag="r")
                    nc.vector.reciprocal(out=r, in_=o_ps[:, :, H])
                    ob = ob0 + pr * 2
                    nc.vector.tensor_scalar_mul(
                        out=o_sb[:, ob, :], in0=o_ps[:, 0, 0:H],
                        scalar1=r[:, 0:1],
                    )
                    nc.scalar.mul(out=o_sb[:, ob + 1, :], in_=o_ps[:, 1, 0:H],
                                  mul=r[:, 1:2])
                if ob0 + QUAD == G:
                    nc.sync.dma_start(
                        out=o[:, gb * G : (gb + 1) * G, :], in_=o_sb
                    )
                    del osb_t[gb]

            # gT evacuation for iteration 0 (no wei block ran)
            if Q == 0:
                gsb = pgsb.tile([128, 1024], BF16, tag="gsb")
                nc.gpsimd.memset(gsb[64:128, 512:1024], 0.0)
                nc.scalar.copy(out=gsb[:, 0:512], in_=gt_t[:, 0:512])
                nc.vector.tensor_copy(out=gsb[0:64, 512:1024],
                                      in_=gt_t[0:64, 512:1024])
                gsb_t[Q] = gsb
    return nc


_cached = {}


def _get_nc(nb):
    if nb not in _cached:
        _cached[nb] = build_nc(nb)
    return _cached[nb]


def prep_inputs(x, Wq, Wk, Wv, nb=NB, ncores=NCORES):
    """Host-side sharding + layout/dtype prep + weight folding + zero-pad."""
    x = np.asarray(x, dtype=np.float32)
    A = (np.asarray(Wq, np.float32).T @ np.asarray(Wk, np.float32)) * SCALE
    a_bf = np.zeros((EP, EP), dtype=NPBF16)
    a_bf[0:E, 0:E] = A.astype(NPBF16)
    wvt_bf = np.zeros((EP, H), dtype=NPBF16)
    wvt_bf[0:E] = np.ascontiguousarray(np.asarray(Wv, np.float32).T).astype(NPBF16)
    in_maps = []
    for c in range(ncores):
        shard = x[c * nb : (c + 1) * nb]                      # [nb, T, E]
        xt = np.zeros((EP, nb * T), dtype=NPBF16)
        xt[0:E] = (
            np.ascontiguousarray(shard.transpose(2, 0, 1))
            .reshape(E, nb * T)
            .astype(NPBF16)
        )
        in_maps.append({"xt": xt, "a": a_bf, "wvt": wvt_bf})
    return in_maps


def kernel(x, Wq, Wk, Wv, _trace=False):
    nc = _get_nc(NB)
    in_maps = prep_inputs(x, Wq, Wk, Wv)
    res = run_bass_kernel_spmd(
        nc, in_maps, core_ids=list(range(NCORES)), trace=_trace
    )
    # o is [T, nb, H] per core; transpose to [nb, T, H] and concat.
    out = np.concatenate(
        [np.asarray(res.results[c]["o"], dtype=np.float32).transpose(1, 0, 2)
         for c in range(NCORES)], axis=0
    )
    out = np.ascontiguousarray(out, dtype=np.float32)
    if _trace:
        kernel.last_result = res
    return out
